# revision 33
# baseline (speedup 1.0000x reference)
"""Cosine-similarity loss on Trainium2 — 8-core SPMD Bass/Tile kernel (v13).

Math (per token, logits row l of length V, target t):
    probs = softmax(l);  cos = probs[t] / ||probs||_2
  The softmax normalizer cancels in the ratio:
    cos = exp(l_t) / sqrt(sum_i exp(2*l_i))
  loss = 1 - sum(cos * mask) / (sum(mask) + 1e-8),  mask = (t != 0)

Two-path vocab-sum over fp8e4m3-staged logits (16.4 MB/core):
  * ACT share (VA cols, token-major): native Exp at 1 elem/cycle/lane
    @1.2GHz with free fp32 accumulation (accum_out).  ~150 G elem/s.
  * PE share (VP rows, vocab-major, staged transposed on host): one
    2x-mode DVE tensor_scalar makes int16(l*A16+B16) whose bit pattern
    IS exp(2l) in bf16 (~243 G elem/s); the TensorEngine reduces along
    partitions (= vocab) via ones[128,1] matmuls accumulating into one
    PSUM row [1, 512tok] at 215 ns per 512-col MM (warm).

Scheduling (lessons from v5-v9 traces):
  * ONE HWDGE ring (nc.sync), each chunk's DMA issued at its consumption
    point in deadline order; pool slot-semaphores keep the stream ~2-3
    chunks ahead.  Front-loading all 16.4MB instead throttles ACT/DVE
    ~20% via SBUF-bank contention; scalar-ring triggers eat ~0.7us of
    ACT queue time each.
  * First chunks of both streams are small so engines start by ~9us;
    last PE chunks small to cut post-DMA trailing latency.
  * exp_lt / em pinned after the bulk streams with explicit ordering
    deps — the scheduler otherwise hoists them onto the engine FIFO
    heads where their gather-dependency blocks the queue for ~10us.
  * Tail: PSUM row -> bf16 SBUF -> 4 one-pass bf16 transpose matmuls ->
    s2 add; rsqrt = fast-inverse-sqrt bit trick WITHOUT the Newton step
    (~6e-5 on the loss; tol 2e-2); numerator pre-masked; mask-sum
    reduced early.
  * Numerator l_t host-gathered (full fp32 precision) into the packed
    aux input — on-device indirect gathers completed as late as ~69us
    on throttled runs (SWDGE descgen locked out by DVE 2-port ts) and
    gated the output.  Mask from gidx != token_index*V (both in aux).

Sharding: tokens (B*S = 4096) split evenly across 8 NeuronCores, 512/core
(4 tiles of 128 partitions, token j at partition j%128, tile j//128).
Each core returns per-partition partials of cos*mask and mask; the host
adds 8x128 partials and finishes the division.
"""

import numpy as np
import ml_dtypes

import concourse.bacc as bacc
import concourse.bass as bass
import concourse.mybir as mybir
import concourse.tile as tile
from concourse.tile import add_dep_helper
from concourse.bass_utils import run_bass_kernel_spmd

B, S, V = 2, 2048, 32000
N_CORES = 8
NTOK = B * S                      # 4096
TOK_PER_CORE = NTOK // N_CORES    # 512
P = 128
TILES = TOK_PER_CORE // P         # 4 token tiles per core
EPS_MEAN = 1e-8

# vocab split between the two paths
VA = 13824                        # ACT share (token-major)
# ACT chunks as (tile_row, col0, width): first chunk small so ACT starts
# early; one accum column per chunk, rows 0/3 split for small slivers.
# v18 traces showed ACT idle ~12us at the end while DVE churned: the
# split should equalize FINISH times, not busy times -> ACT gets more.
A_CHUNKS = [(0, 0, 2048), (0, 2048, 11776),
            (1, 0, 13824), (2, 0, 13824),
            (3, 0, 11776), (3, 11776, 2048)]
VP = V - VA                       # 18176 PE share (vocab-major)
NP = VP // P                      # 142 vocab tiles of 128
# vocab tiles per chunk; small first (early DVE start) and small last
# (short trailing latency after the final DMA)
PE_CHUNKS = [10] + [17] * 7 + [7, 6]
assert sum(PE_CHUNKS) == NP
# single-ring issue order, sorted by when each chunk is consumed
ISSUE_ORDER = [("A", 0), ("P", 0), ("G", 0), ("A", 1), ("P", 1), ("P", 2),
               ("A", 2), ("P", 3), ("P", 4), ("A", 3), ("P", 5), ("P", 6),
               ("A", 4), ("P", 7), ("P", 8), ("P", 9), ("A", 5)]

# Schraudolph constants for exp(2*l) in the int16/bf16 domain:
#   bits16 = round((2*l) * (2^23/ln2)/2^16 + (127*2^23 - C)/2^16)
SCHRAUD_C = 366393.0
A16 = 2.0 * float(1 << 23) / float(np.log(2.0)) / 65536.0
B16 = (127.0 * float(1 << 23) - SCHRAUD_C) / 65536.0 - 4.04  # -4.04: bias trim


def build_program():
    """Build + compile the per-core Bass program (identical on all cores)."""
    # NOTE: no num_devices — per-core programs are fully independent (the host
    # combines partials); num_devices>1 makes Tile emit a cross-device exit
    # barrier that crashes under the axon PJRT shim.
    nc = bacc.Bacc("TRN2", target_bir_lowering=False, debug=False)
    f32 = mybir.dt.float32
    i32 = mybir.dt.int32
    i16 = mybir.dt.int16
    bf16 = mybir.dt.bfloat16
    fp8 = mybir.dt.float8e4
    AF = mybir.ActivationFunctionType
    ALU = mybir.AluOpType
    AX = mybir.AxisListType

    l8a = nc.dram_tensor("l8a", [TOK_PER_CORE, VA], fp8, kind="ExternalInput").ap()
    l8p = nc.dram_tensor("l8p", [P, NP * TOK_PER_CORE], fp8, kind="ExternalInput").ap()
    # aux cols: [gidx(4) | gbase(4) | ltg-bits(4)] — target flat index,
    # token-index*V, and the host-gathered fp32 target logits (bitcast).
    aux = nc.dram_tensor("aux", [P, 3 * TILES], i32, kind="ExternalInput").ap()
    out = nc.dram_tensor("out", [P, 2], f32, kind="ExternalOutput").ap()

    with tile.TileContext(nc) as tc:
        with (
            tc.tile_pool(name="adata", bufs=3) as adata,
            tc.tile_pool(name="pdata", bufs=3) as pdata,
            tc.tile_pool(name="ywork", bufs=2) as ywork,
            tc.tile_pool(name="small", bufs=1) as small,
            tc.tile_pool(name="psacc", bufs=1, space="PSUM") as psacc,
            tc.tile_pool(name="pstr", bufs=1, space="PSUM") as pstr,
        ):
            s2a = small.tile([P, len(A_CHUNKS)], f32)
            res = small.tile([P, 2], f32)

            # PSUM accumulator row: per-token sum of exp(2l) over the PE share
            ps_row = psacc.tile([1, TOK_PER_CORE], f32)

            # stationary ones for the PE vocab reduction (bf16 for 1-pass MMs)
            ones_bf = small.tile([P, 1], bf16)
            nc.any.memset(ones_bf[:], 1.0)
            ones_b1 = small.tile([1, 1], bf16)
            nc.any.memset(ones_b1[:], 1.0)

            # --- self-clocked stream: each chunk's DMA issued at its
            # consumption point in deadline order on ONE ring.  The sync
            # queue races ahead so DMAs stay ~bufs chunks ahead of compute;
            # slot semaphores throttle the stream to consumption rate.
            # (Front-loading all 16.4MB instead slams SBUF and throttles
            # ACT/DVE ~20% — the v8/v9 regression.)
            a_tiles = {}
            p_tiles = {}
            aux_sb = small.tile([P, 3 * TILES], i32)

            def issue(kind, idx):
                if kind == "G":
                    nc.sync.dma_start(out=aux_sb[:], in_=aux)
                elif kind == "A":
                    t, c0, w = A_CHUNKS[idx]
                    ach = adata.tile([P, 13824], fp8, tag="achunk")
                    nc.sync.dma_start(
                        out=ach[:, :w], in_=l8a[t * P : (t + 1) * P, c0 : c0 + w]
                    )
                    a_tiles[idx] = ach
                else:
                    ntile = PE_CHUNKS[idx]
                    col0 = sum(PE_CHUNKS[:idx]) * TOK_PER_CORE
                    pch = pdata.tile([P, 18 * TOK_PER_CORE], fp8, tag="pchunk")
                    nc.sync.dma_start(
                        out=pch[:, : ntile * TOK_PER_CORE],
                        in_=l8p[:, col0 : col0 + ntile * TOK_PER_CORE],
                    )
                    p_tiles[idx] = pch

            # first chunks + gidx issued before the small setup ops
            for ev in ISSUE_ORDER[:3]:
                issue(*ev)

            # mask: pad token <=> target==0 <=> gidx == token_index*V
            # (both staged in aux; v10-12 lesson: the on-device indirect
            # gathers for the numerator could complete as late as ~69us on
            # throttled runs — SWDGE descgen is locked out by DVE 2-port ts
            # instructions — so l_t is host-gathered into aux instead).
            mask_sb = small.tile([P, TILES], f32)
            nc.vector.tensor_tensor(
                out=mask_sb[:], in0=aux_sb[:, 0:TILES],
                in1=aux_sb[:, TILES : 2 * TILES], op=ALU.not_equal
            )
            # mask-sum is independent of everything else: do it now
            nc.vector.tensor_reduce(
                out=res[:, 1:2], in_=mask_sb[:], axis=AX.X, op=ALU.add
            )

            # --- main loop: issue remaining DMAs at their deadline slot,
            # compute each chunk as it lands.
            state = {"acts": {}, "tss": {}, "mm_done": 0}

            def compute(kind, idx):
                if kind == "G":
                    return
                if kind == "A":
                    ach = a_tiles[idx]
                    t, c0, w = A_CHUNKS[idx]
                    # in-place fp8 out is clamped garbage nothing reads; the
                    # accumulated fp32 row sums are the real output.
                    state["acts"][idx] = nc.scalar.activation(
                        out=ach[:, :w], in_=ach[:, :w], func=AF.Exp, scale=2.0,
                        accum_out=s2a[:, idx : idx + 1],
                    )
                else:
                    ntile = PE_CHUNKS[idx]
                    pch = p_tiles[idx]
                    y16 = ywork.tile([P, 18 * TOK_PER_CORE], i16, tag="y16")
                    yb = y16[:].bitcast(bf16)
                    # last chunk: split the ts in two so its first MMs start
                    # ~1.3us earlier, shortening the MM tail before the drain
                    halves = ([(0, ntile)] if idx != len(PE_CHUNKS) - 1
                              else [(0, 5), (5, ntile)])
                    for t0, t1 in halves:
                        c0, c1 = t0 * TOK_PER_CORE, t1 * TOK_PER_CORE
                        state["tss"][idx] = nc.vector.tensor_scalar(
                            out=y16[:, c0:c1], in0=pch[:, c0:c1],
                            scalar1=float(A16), scalar2=float(B16),
                            op0=ALU.mult, op1=ALU.add,
                        )
                        for k in range(t0, t1):
                            nc.tensor.matmul(
                                ps_row[:1, :],
                                ones_bf[:],
                                yb[:, k * TOK_PER_CORE : (k + 1) * TOK_PER_CORE],
                                start=(state["mm_done"] == 0),
                                stop=(state["mm_done"] == NP - 1),
                            )
                            state["mm_done"] += 1

            for n, ev in enumerate(ISSUE_ORDER):
                if n >= 3:
                    issue(*ev)
                compute(*ev)

            # --- PE-share drain: PSUM row -> bf16 SBUF -> token-major
            # [128, TILES] via 4 tiny 1-pass bf16 transpose matmuls.
            s2row = small.tile([1, TOK_PER_CORE], bf16)
            nc.vector.tensor_copy(s2row[:], ps_row[:1, :])
            ps_t = pstr.tile([P, TILES], f32)
            for t in range(TILES):
                nc.tensor.matmul(
                    ps_t[:, t : t + 1],
                    s2row[:1, t * P : (t + 1) * P],
                    ones_b1[:1, :],
                    start=True, stop=True,
                )
            s2p = small.tile([P, TILES], f32)
            nc.vector.tensor_copy(s2p[:], ps_t[:])

            # --- numerator exp, pre-masked.  Anchored MID-stream (after
            # A1 / ts3): their inputs are ready by ~9us (aux host-staged),
            # so they slot into the engines' starve gaps instead of
            # serializing after the bulk streams; anchoring them at all
            # keeps them off the FIFO heads (the v8 lesson).
            exp_lt = small.tile([P, TILES], f32)
            ei = nc.scalar.activation(
                out=exp_lt[:], in_=aux_sb[:, 2 * TILES : 3 * TILES].bitcast(f32),
                func=AF.Exp,
            )
            add_dep_helper(ei.ins, state["acts"][1].ins, sync=False,
                           reason="exp_lt mid ACT stream")
            em = small.tile([P, TILES], f32)
            emi = nc.vector.tensor_mul(em[:], exp_lt[:], mask_sb[:])
            add_dep_helper(emi.ins, state["tss"][3].ins, sync=False,
                           reason="em mid ts stream")

            # fold the split rows' accum chunks: col1 += col0 (row 0, runs
            # mid-stream — A0/A1 accums ready ~22us) and col4 += col5 (row 3
            # — A5's accum is late, so anchor after the last ts to keep it
            # off the DVE FIFO head).  s2a[:, 1:5] is then [r0, r1, r2, r3].
            fi = nc.vector.tensor_add(s2a[:, 1:2], s2a[:, 0:1], s2a[:, 1:2])
            add_dep_helper(fi.ins, state["tss"][4].ins, sync=False,
                           reason="s2a fold row0 mid ts stream")
            f2 = nc.vector.tensor_add(s2a[:, 4:5], s2a[:, 5:6], s2a[:, 4:5])
            add_dep_helper(f2.ins, state["tss"][len(PE_CHUNKS) - 1].ins,
                           sync=False, reason="s2a fold row3 after ts stream")
            s2 = small.tile([P, TILES], f32)
            nc.vector.tensor_add(s2[:], s2a[:, 1 : 1 + TILES], s2p[:])

            # rs ~= 1/sqrt(s2): fast-inverse-sqrt bit trick, no Newton step
            # (y0 rel err in [-3.4%, +1.2%] -> ~6e-5 on the loss; tol 2e-2).
            sh = small.tile([P, TILES], i32)
            nc.vector.tensor_scalar(
                out=sh[:], in0=s2[:].bitcast(i32), scalar1=1, scalar2=None,
                op0=ALU.arith_shift_right,
            )
            y0i = small.tile([P, TILES], i32)
            nc.vector.tensor_scalar(
                out=y0i[:], in0=sh[:], scalar1=-1.0, scalar2=float(0x5F3759DF),
                op0=ALU.mult, op1=ALU.add,
            )
            cosm = small.tile([P, TILES], f32)
            nc.vector.tensor_mul(cosm[:], em[:], y0i[:].bitcast(f32))

            nc.vector.tensor_reduce(
                out=res[:, 0:1], in_=cosm[:], axis=AX.X, op=ALU.add
            )
            nc.sync.dma_start(out=out, in_=res[:])

    nc.compile()
    return nc


_NC_CACHE = {}


def _get_nc():
    if "nc" not in _NC_CACHE:
        _NC_CACHE["nc"] = build_program()
    return _NC_CACHE["nc"]


def make_in_maps(logits, targets):
    """Shard full inputs into per-core input maps (host-side prep only)."""
    logits = np.asarray(logits)
    targets = np.asarray(targets)
    assert logits.shape == (B, S, V), logits.shape
    lf = np.ascontiguousarray(logits.reshape(NTOK, V).astype(np.float32, copy=False))
    l8f = lf.astype(ml_dtypes.float8_e4m3fn)
    tf = targets.reshape(NTOK).astype(np.int64)

    # token j of a core sits at (partition p = j % P, tile t = j // P)
    local_tok = (np.arange(TILES)[None, :] * P + np.arange(P)[:, None]).astype(np.int64)

    in_maps = []
    for k in range(N_CORES):
        sl = slice(k * TOK_PER_CORE, (k + 1) * TOK_PER_CORE)
        blk = lf[sl]                              # [512, V] fp32
        tk = tf[sl].reshape(TILES, P).T           # [P, TILES]
        gidx = (local_tok * V + tk).astype(np.int32)
        gbase = (local_tok * V).astype(np.int32)
        ltg = blk[local_tok, tk].astype(np.float32)   # host-gathered l_t
        aux = np.concatenate(
            [gidx, gbase, ltg.view(np.int32)], axis=1
        ).astype(np.int32)
        blk8 = l8f[sl]                            # [512, V]
        # PE share staged vocab-major: l8p[p, j*512+t] = l[t, VA + j*128 + p]
        l8p = np.ascontiguousarray(
            blk8[:, VA:].reshape(TOK_PER_CORE, NP, P).transpose(2, 1, 0)
            .reshape(P, NP * TOK_PER_CORE)
        )
        in_maps.append(
            {
                "l8a": np.ascontiguousarray(blk8[:, :VA]),
                "l8p": l8p,
                "aux": np.ascontiguousarray(aux),
            }
        )
    return in_maps


def reduce_outputs(per_core_outs):
    """Combine per-core [128, 2] partials into the final scalar loss."""
    s = 0.0
    c = 0.0
    for o in per_core_outs:
        s += float(o[:, 0].astype(np.float64).sum())
        c += float(o[:, 1].astype(np.float64).sum())
    return np.asarray(np.float32(1.0 - s / (c + EPS_MEAN)))


def run_on_device(in_maps, **kwargs):
    nc = _get_nc()
    return run_bass_kernel_spmd(nc, in_maps, core_ids=list(range(N_CORES)), **kwargs)


def kernel(logits, targets):
    in_maps = make_in_maps(logits, targets)
    res = run_on_device(in_maps)
    return reduce_outputs([r["out"] for r in res.results])


# revision 35
# speedup vs baseline: 1.0227x; 1.0227x over previous
"""Cosine-similarity loss on Trainium2 — 8-core SPMD Bass/Tile kernel (v13).

Math (per token, logits row l of length V, target t):
    probs = softmax(l);  cos = probs[t] / ||probs||_2
  The softmax normalizer cancels in the ratio:
    cos = exp(l_t) / sqrt(sum_i exp(2*l_i))
  loss = 1 - sum(cos * mask) / (sum(mask) + 1e-8),  mask = (t != 0)

Two-path vocab-sum over fp8e4m3-staged logits (16.4 MB/core):
  * ACT share (VA cols, token-major): native Exp at 1 elem/cycle/lane
    @1.2GHz with free fp32 accumulation (accum_out).  ~150 G elem/s.
  * PE share (VP rows, vocab-major, staged transposed on host): one
    2x-mode DVE tensor_scalar makes int16(l*A16+B16) whose bit pattern
    IS exp(2l) in bf16 (~243 G elem/s); the TensorEngine reduces along
    partitions (= vocab) via ones[128,1] matmuls accumulating into one
    PSUM row [1, 512tok] at 215 ns per 512-col MM (warm).

Scheduling (lessons from v5-v9 traces):
  * ONE HWDGE ring (nc.sync), each chunk's DMA issued at its consumption
    point in deadline order; pool slot-semaphores keep the stream ~2-3
    chunks ahead.  Front-loading all 16.4MB instead throttles ACT/DVE
    ~20% via SBUF-bank contention; scalar-ring triggers eat ~0.7us of
    ACT queue time each.
  * First chunks of both streams are small so engines start by ~9us;
    last PE chunks small to cut post-DMA trailing latency.
  * exp_lt / em pinned after the bulk streams with explicit ordering
    deps — the scheduler otherwise hoists them onto the engine FIFO
    heads where their gather-dependency blocks the queue for ~10us.
  * Tail: PSUM row -> bf16 SBUF -> 4 one-pass bf16 transpose matmuls ->
    s2 add; rsqrt = fast-inverse-sqrt bit trick WITHOUT the Newton step
    (~6e-5 on the loss; tol 2e-2); numerator pre-masked; mask-sum
    reduced early.
  * Numerator l_t host-gathered (full fp32 precision) into the packed
    aux input — on-device indirect gathers completed as late as ~69us
    on throttled runs (SWDGE descgen locked out by DVE 2-port ts) and
    gated the output.  Mask from gidx != token_index*V (both in aux).

Sharding: tokens (B*S = 4096) split evenly across 8 NeuronCores, 512/core
(4 tiles of 128 partitions, token j at partition j%128, tile j//128).
Each core returns per-partition partials of cos*mask and mask; the host
adds 8x128 partials and finishes the division.
"""

import numpy as np
import ml_dtypes

import concourse.bacc as bacc
import concourse.bass as bass
import concourse.mybir as mybir
import concourse.tile as tile
from concourse.tile import add_dep_helper
from concourse.bass_utils import run_bass_kernel_spmd

B, S, V = 2, 2048, 32000
N_CORES = 8
NTOK = B * S                      # 4096
TOK_PER_CORE = NTOK // N_CORES    # 512
P = 128
TILES = TOK_PER_CORE // P         # 4 token tiles per core
EPS_MEAN = 1e-8

# vocab split between the two paths
VA = 13568                        # ACT share (token-major)
# ACT chunks as (tile_row, col0, width): first chunk small so ACT starts
# early; one accum column per chunk, rows 0/3 split for small slivers.
# v18 traces showed ACT idle ~12us at the end while DVE churned: the
# split should equalize FINISH times, not busy times -> ACT gets more.
A_CHUNKS = [(0, 0, 2048), (0, 2048, 11520),
            (1, 0, 13568), (2, 0, 13568),
            (3, 0, 11520), (3, 11520, 2048)]
VP = V - VA                       # 18432 PE share (vocab-major)
NP = VP // P                      # 144 vocab tiles of 128
# vocab tiles per chunk; small first (early DVE start) and small last
# (short trailing latency after the final DMA)
PE_CHUNKS = [10] + [17] * 7 + [8, 7]
assert sum(PE_CHUNKS) == NP
# single-ring issue order, sorted by when each chunk is consumed
ISSUE_ORDER = [("A", 0), ("P", 0), ("G", 0), ("A", 1), ("P", 1), ("P", 2),
               ("A", 2), ("P", 3), ("P", 4), ("A", 3), ("P", 5), ("P", 6),
               ("A", 4), ("P", 7), ("P", 8), ("P", 9), ("A", 5)]

# Schraudolph constants for exp(2*l) in the int16/bf16 domain:
#   bits16 = round((2*l) * (2^23/ln2)/2^16 + (127*2^23 - C)/2^16)
SCHRAUD_C = 366393.0
A16 = 2.0 * float(1 << 23) / float(np.log(2.0)) / 65536.0
B16 = (127.0 * float(1 << 23) - SCHRAUD_C) / 65536.0 - 4.04  # -4.04: bias trim


def build_program():
    """Build + compile the per-core Bass program (identical on all cores)."""
    # NOTE: no num_devices — per-core programs are fully independent (the host
    # combines partials); num_devices>1 makes Tile emit a cross-device exit
    # barrier that crashes under the axon PJRT shim.
    nc = bacc.Bacc("TRN2", target_bir_lowering=False, debug=False)
    f32 = mybir.dt.float32
    i32 = mybir.dt.int32
    i16 = mybir.dt.int16
    bf16 = mybir.dt.bfloat16
    fp8 = mybir.dt.float8e4
    AF = mybir.ActivationFunctionType
    ALU = mybir.AluOpType
    AX = mybir.AxisListType

    l8a = nc.dram_tensor("l8a", [TOK_PER_CORE, VA], fp8, kind="ExternalInput").ap()
    l8p = nc.dram_tensor("l8p", [P, NP * TOK_PER_CORE], fp8, kind="ExternalInput").ap()
    # aux cols: [gidx(4) | gbase(4) | ltg-bits(4)] — target flat index,
    # token-index*V, and the host-gathered fp32 target logits (bitcast).
    aux = nc.dram_tensor("aux", [P, 3 * TILES], i32, kind="ExternalInput").ap()
    out = nc.dram_tensor("out", [P, 2], f32, kind="ExternalOutput").ap()

    with tile.TileContext(nc) as tc:
        with (
            tc.tile_pool(name="adata", bufs=3) as adata,
            tc.tile_pool(name="pdata", bufs=3) as pdata,
            tc.tile_pool(name="ywork", bufs=2) as ywork,
            tc.tile_pool(name="small", bufs=1) as small,
            tc.tile_pool(name="psacc", bufs=1, space="PSUM") as psacc,
            tc.tile_pool(name="pstr", bufs=1, space="PSUM") as pstr,
        ):
            s2a = small.tile([P, len(A_CHUNKS)], f32)
            res = small.tile([P, 2], f32)

            # PSUM accumulator row: per-token sum of exp(2l) over the PE share
            ps_row = psacc.tile([1, TOK_PER_CORE], f32)

            # stationary ones for the PE vocab reduction (bf16 for 1-pass MMs)
            ones_bf = small.tile([P, 1], bf16)
            nc.any.memset(ones_bf[:], 1.0)
            ones_b1 = small.tile([1, 1], bf16)
            nc.any.memset(ones_b1[:], 1.0)

            # --- self-clocked stream: each chunk's DMA issued at its
            # consumption point in deadline order on ONE ring.  The sync
            # queue races ahead so DMAs stay ~bufs chunks ahead of compute;
            # slot semaphores throttle the stream to consumption rate.
            # (Front-loading all 16.4MB instead slams SBUF and throttles
            # ACT/DVE ~20% — the v8/v9 regression.)
            a_tiles = {}
            p_tiles = {}
            aux_sb = small.tile([P, 3 * TILES], i32)

            def issue(kind, idx):
                if kind == "G":
                    nc.sync.dma_start(out=aux_sb[:], in_=aux)
                elif kind == "A":
                    t, c0, w = A_CHUNKS[idx]
                    ach = adata.tile([P, 13568], fp8, tag="achunk")
                    nc.sync.dma_start(
                        out=ach[:, :w], in_=l8a[t * P : (t + 1) * P, c0 : c0 + w]
                    )
                    a_tiles[idx] = ach
                else:
                    ntile = PE_CHUNKS[idx]
                    col0 = sum(PE_CHUNKS[:idx]) * TOK_PER_CORE
                    pch = pdata.tile([P, 18 * TOK_PER_CORE], fp8, tag="pchunk")
                    nc.sync.dma_start(
                        out=pch[:, : ntile * TOK_PER_CORE],
                        in_=l8p[:, col0 : col0 + ntile * TOK_PER_CORE],
                    )
                    p_tiles[idx] = pch

            # first chunks + gidx issued before the small setup ops
            for ev in ISSUE_ORDER[:3]:
                issue(*ev)

            # mask: pad token <=> target==0 <=> gidx == token_index*V
            # (both staged in aux; v10-12 lesson: the on-device indirect
            # gathers for the numerator could complete as late as ~69us on
            # throttled runs — SWDGE descgen is locked out by DVE 2-port ts
            # instructions — so l_t is host-gathered into aux instead).
            mask_sb = small.tile([P, TILES], f32)
            nc.vector.tensor_tensor(
                out=mask_sb[:], in0=aux_sb[:, 0:TILES],
                in1=aux_sb[:, TILES : 2 * TILES], op=ALU.not_equal
            )
            # mask-sum is independent of everything else: do it now
            nc.vector.tensor_reduce(
                out=res[:, 1:2], in_=mask_sb[:], axis=AX.X, op=ALU.add
            )

            # --- main loop: issue remaining DMAs at their deadline slot,
            # compute each chunk as it lands.
            state = {"acts": {}, "tss": {}, "mm_done": 0}

            def compute(kind, idx):
                if kind == "G":
                    return
                if kind == "A":
                    ach = a_tiles[idx]
                    t, c0, w = A_CHUNKS[idx]
                    # in-place fp8 out is clamped garbage nothing reads; the
                    # accumulated fp32 row sums are the real output.
                    state["acts"][idx] = nc.scalar.activation(
                        out=ach[:, :w], in_=ach[:, :w], func=AF.Exp, scale=2.0,
                        accum_out=s2a[:, idx : idx + 1],
                    )
                else:
                    ntile = PE_CHUNKS[idx]
                    pch = p_tiles[idx]
                    y16 = ywork.tile([P, 18 * TOK_PER_CORE], i16, tag="y16")
                    yb = y16[:].bitcast(bf16)
                    # last chunk: split the ts in two so its first MMs start
                    # ~1.3us earlier, shortening the MM tail before the drain
                    halves = ([(0, ntile)] if idx != len(PE_CHUNKS) - 1
                              else [(0, 5), (5, ntile)])
                    for t0, t1 in halves:
                        c0, c1 = t0 * TOK_PER_CORE, t1 * TOK_PER_CORE
                        state["tss"][idx] = nc.vector.tensor_scalar(
                            out=y16[:, c0:c1], in0=pch[:, c0:c1],
                            scalar1=float(A16), scalar2=float(B16),
                            op0=ALU.mult, op1=ALU.add,
                        )
                        for k in range(t0, t1):
                            nc.tensor.matmul(
                                ps_row[:1, :],
                                ones_bf[:],
                                yb[:, k * TOK_PER_CORE : (k + 1) * TOK_PER_CORE],
                                start=(state["mm_done"] == 0),
                                stop=(state["mm_done"] == NP - 1),
                            )
                            state["mm_done"] += 1

            for n, ev in enumerate(ISSUE_ORDER):
                if n >= 3:
                    issue(*ev)
                compute(*ev)

            # --- PE-share drain: PSUM row -> bf16 SBUF -> token-major
            # [128, TILES] via 4 tiny 1-pass bf16 transpose matmuls.
            s2row = small.tile([1, TOK_PER_CORE], bf16)
            nc.vector.tensor_copy(s2row[:], ps_row[:1, :])
            ps_t = pstr.tile([P, TILES], f32)
            for t in range(TILES):
                nc.tensor.matmul(
                    ps_t[:, t : t + 1],
                    s2row[:1, t * P : (t + 1) * P],
                    ones_b1[:1, :],
                    start=True, stop=True,
                )
            s2p = small.tile([P, TILES], f32)
            nc.vector.tensor_copy(s2p[:], ps_t[:])

            # --- numerator exp, pre-masked.  Anchored MID-stream (after
            # A1 / ts3): their inputs are ready by ~9us (aux host-staged),
            # so they slot into the engines' starve gaps instead of
            # serializing after the bulk streams; anchoring them at all
            # keeps them off the FIFO heads (the v8 lesson).
            exp_lt = small.tile([P, TILES], f32)
            ei = nc.scalar.activation(
                out=exp_lt[:], in_=aux_sb[:, 2 * TILES : 3 * TILES].bitcast(f32),
                func=AF.Exp,
            )
            add_dep_helper(ei.ins, state["acts"][1].ins, sync=False,
                           reason="exp_lt mid ACT stream")
            em = small.tile([P, TILES], f32)
            emi = nc.vector.tensor_mul(em[:], exp_lt[:], mask_sb[:])
            add_dep_helper(emi.ins, state["tss"][3].ins, sync=False,
                           reason="em mid ts stream")

            # fold the split rows' accum chunks: col1 += col0 (row 0, runs
            # mid-stream — A0/A1 accums ready ~22us) and col4 += col5 (row 3
            # — A5's accum is late, so anchor after the last ts to keep it
            # off the DVE FIFO head).  s2a[:, 1:5] is then [r0, r1, r2, r3].
            fi = nc.vector.tensor_add(s2a[:, 1:2], s2a[:, 0:1], s2a[:, 1:2])
            add_dep_helper(fi.ins, state["tss"][4].ins, sync=False,
                           reason="s2a fold row0 mid ts stream")
            f2 = nc.vector.tensor_add(s2a[:, 4:5], s2a[:, 5:6], s2a[:, 4:5])
            add_dep_helper(f2.ins, state["tss"][len(PE_CHUNKS) - 1].ins,
                           sync=False, reason="s2a fold row3 after ts stream")
            s2 = small.tile([P, TILES], f32)
            nc.vector.tensor_add(s2[:], s2a[:, 1 : 1 + TILES], s2p[:])

            # rs ~= 1/sqrt(s2): fast-inverse-sqrt bit trick, no Newton step
            # (y0 rel err in [-3.4%, +1.2%] -> ~6e-5 on the loss; tol 2e-2).
            sh = small.tile([P, TILES], i32)
            nc.vector.tensor_scalar(
                out=sh[:], in0=s2[:].bitcast(i32), scalar1=1, scalar2=None,
                op0=ALU.arith_shift_right,
            )
            y0i = small.tile([P, TILES], i32)
            nc.vector.tensor_scalar(
                out=y0i[:], in0=sh[:], scalar1=-1.0, scalar2=float(0x5F3759DF),
                op0=ALU.mult, op1=ALU.add,
            )
            cosm = small.tile([P, TILES], f32)
            nc.vector.tensor_mul(cosm[:], em[:], y0i[:].bitcast(f32))

            nc.vector.tensor_reduce(
                out=res[:, 0:1], in_=cosm[:], axis=AX.X, op=ALU.add
            )
            nc.sync.dma_start(out=out, in_=res[:])

    nc.compile()
    return nc


_NC_CACHE = {}


def _get_nc():
    if "nc" not in _NC_CACHE:
        _NC_CACHE["nc"] = build_program()
    return _NC_CACHE["nc"]


def make_in_maps(logits, targets):
    """Shard full inputs into per-core input maps (host-side prep only)."""
    logits = np.asarray(logits)
    targets = np.asarray(targets)
    assert logits.shape == (B, S, V), logits.shape
    lf = np.ascontiguousarray(logits.reshape(NTOK, V).astype(np.float32, copy=False))
    l8f = lf.astype(ml_dtypes.float8_e4m3fn)
    tf = targets.reshape(NTOK).astype(np.int64)

    # token j of a core sits at (partition p = j % P, tile t = j // P)
    local_tok = (np.arange(TILES)[None, :] * P + np.arange(P)[:, None]).astype(np.int64)

    in_maps = []
    for k in range(N_CORES):
        sl = slice(k * TOK_PER_CORE, (k + 1) * TOK_PER_CORE)
        blk = lf[sl]                              # [512, V] fp32
        tk = tf[sl].reshape(TILES, P).T           # [P, TILES]
        gidx = (local_tok * V + tk).astype(np.int32)
        gbase = (local_tok * V).astype(np.int32)
        ltg = blk[local_tok, tk].astype(np.float32)   # host-gathered l_t
        aux = np.concatenate(
            [gidx, gbase, ltg.view(np.int32)], axis=1
        ).astype(np.int32)
        blk8 = l8f[sl]                            # [512, V]
        # PE share staged vocab-major: l8p[p, j*512+t] = l[t, VA + j*128 + p]
        l8p = np.ascontiguousarray(
            blk8[:, VA:].reshape(TOK_PER_CORE, NP, P).transpose(2, 1, 0)
            .reshape(P, NP * TOK_PER_CORE)
        )
        in_maps.append(
            {
                "l8a": np.ascontiguousarray(blk8[:, :VA]),
                "l8p": l8p,
                "aux": np.ascontiguousarray(aux),
            }
        )
    return in_maps


def reduce_outputs(per_core_outs):
    """Combine per-core [128, 2] partials into the final scalar loss."""
    s = 0.0
    c = 0.0
    for o in per_core_outs:
        s += float(o[:, 0].astype(np.float64).sum())
        c += float(o[:, 1].astype(np.float64).sum())
    return np.asarray(np.float32(1.0 - s / (c + EPS_MEAN)))


def run_on_device(in_maps, **kwargs):
    nc = _get_nc()
    return run_bass_kernel_spmd(nc, in_maps, core_ids=list(range(N_CORES)), **kwargs)


def kernel(logits, targets):
    in_maps = make_in_maps(logits, targets)
    res = run_on_device(in_maps)
    return reduce_outputs([r["out"] for r in res.results])


# revision 36
# speedup vs baseline: 1.0293x; 1.0065x over previous
"""Cosine-similarity loss on Trainium2 — 8-core SPMD Bass/Tile kernel (v13).

Math (per token, logits row l of length V, target t):
    probs = softmax(l);  cos = probs[t] / ||probs||_2
  The softmax normalizer cancels in the ratio:
    cos = exp(l_t) / sqrt(sum_i exp(2*l_i))
  loss = 1 - sum(cos * mask) / (sum(mask) + 1e-8),  mask = (t != 0)

Two-path vocab-sum over fp8e4m3-staged logits (16.4 MB/core):
  * ACT share (VA cols, token-major): native Exp at 1 elem/cycle/lane
    @1.2GHz with free fp32 accumulation (accum_out).  ~150 G elem/s.
  * PE share (VP rows, vocab-major, staged transposed on host): one
    2x-mode DVE tensor_scalar makes int16(l*A16+B16) whose bit pattern
    IS exp(2l) in bf16 (~243 G elem/s); the TensorEngine reduces along
    partitions (= vocab) via ones[128,1] matmuls accumulating into one
    PSUM row [1, 512tok] at 215 ns per 512-col MM (warm).

Scheduling (lessons from v5-v9 traces):
  * ONE HWDGE ring (nc.sync), each chunk's DMA issued at its consumption
    point in deadline order; pool slot-semaphores keep the stream ~2-3
    chunks ahead.  Front-loading all 16.4MB instead throttles ACT/DVE
    ~20% via SBUF-bank contention; scalar-ring triggers eat ~0.7us of
    ACT queue time each.
  * First chunks of both streams are small so engines start by ~9us;
    last PE chunks small to cut post-DMA trailing latency.
  * exp_lt / em pinned after the bulk streams with explicit ordering
    deps — the scheduler otherwise hoists them onto the engine FIFO
    heads where their gather-dependency blocks the queue for ~10us.
  * Tail: PSUM row -> bf16 SBUF -> 4 one-pass bf16 transpose matmuls ->
    s2 add; rsqrt = fast-inverse-sqrt bit trick WITHOUT the Newton step
    (~6e-5 on the loss; tol 2e-2); numerator pre-masked; mask-sum
    reduced early.
  * Numerator l_t host-gathered (full fp32 precision) into the packed
    aux input — on-device indirect gathers completed as late as ~69us
    on throttled runs (SWDGE descgen locked out by DVE 2-port ts) and
    gated the output.  Mask from gidx != token_index*V (both in aux).

Sharding: tokens (B*S = 4096) split evenly across 8 NeuronCores, 512/core
(4 tiles of 128 partitions, token j at partition j%128, tile j//128).
Each core returns per-partition partials of cos*mask and mask; the host
adds 8x128 partials and finishes the division.
"""

import numpy as np
import ml_dtypes

import concourse.bacc as bacc
import concourse.bass as bass
import concourse.mybir as mybir
import concourse.tile as tile
from concourse.tile import add_dep_helper
from concourse.bass_utils import run_bass_kernel_spmd

B, S, V = 2, 2048, 32000
N_CORES = 8
NTOK = B * S                      # 4096
TOK_PER_CORE = NTOK // N_CORES    # 512
P = 128
TILES = TOK_PER_CORE // P         # 4 token tiles per core
EPS_MEAN = 1e-8

# vocab split between the two paths
VA = 13312                        # ACT share (token-major)
# ACT chunks as (tile_row, col0, width): first chunk small so ACT starts
# early; one accum column per chunk, rows 0/3 split for small slivers.
# v18 traces showed ACT idle ~12us at the end while DVE churned: the
# split should equalize FINISH times, not busy times -> ACT gets more.
A_CHUNKS = [(0, 0, 2048), (0, 2048, 11264),
            (1, 0, 13312), (2, 0, 13312),
            (3, 0, 11264), (3, 11264, 2048)]
VP = V - VA                       # 18688 PE share (vocab-major)
NP = VP // P                      # 146 vocab tiles of 128
# vocab tiles per chunk; small first (early DVE start) and small last
# (short trailing latency after the final DMA)
PE_CHUNKS = [10] + [17] * 7 + [9, 8]
assert sum(PE_CHUNKS) == NP
# single-ring issue order, sorted by when each chunk is consumed
ISSUE_ORDER = [("A", 0), ("P", 0), ("G", 0), ("A", 1), ("P", 1), ("P", 2),
               ("A", 2), ("P", 3), ("P", 4), ("A", 3), ("P", 5), ("P", 6),
               ("A", 4), ("P", 7), ("P", 8), ("P", 9), ("A", 5)]

# Schraudolph constants for exp(2*l) in the int16/bf16 domain:
#   bits16 = round((2*l) * (2^23/ln2)/2^16 + (127*2^23 - C)/2^16)
SCHRAUD_C = 366393.0
A16 = 2.0 * float(1 << 23) / float(np.log(2.0)) / 65536.0
B16 = (127.0 * float(1 << 23) - SCHRAUD_C) / 65536.0 - 4.04  # -4.04: bias trim


def build_program():
    """Build + compile the per-core Bass program (identical on all cores)."""
    # NOTE: no num_devices — per-core programs are fully independent (the host
    # combines partials); num_devices>1 makes Tile emit a cross-device exit
    # barrier that crashes under the axon PJRT shim.
    nc = bacc.Bacc("TRN2", target_bir_lowering=False, debug=False)
    f32 = mybir.dt.float32
    i32 = mybir.dt.int32
    i16 = mybir.dt.int16
    bf16 = mybir.dt.bfloat16
    fp8 = mybir.dt.float8e4
    AF = mybir.ActivationFunctionType
    ALU = mybir.AluOpType
    AX = mybir.AxisListType

    l8a = nc.dram_tensor("l8a", [TOK_PER_CORE, VA], fp8, kind="ExternalInput").ap()
    l8p = nc.dram_tensor("l8p", [P, NP * TOK_PER_CORE], fp8, kind="ExternalInput").ap()
    # aux cols: [gidx(4) | gbase(4) | ltg-bits(4)] — target flat index,
    # token-index*V, and the host-gathered fp32 target logits (bitcast).
    aux = nc.dram_tensor("aux", [P, 3 * TILES], i32, kind="ExternalInput").ap()
    out = nc.dram_tensor("out", [P, 2], f32, kind="ExternalOutput").ap()

    with tile.TileContext(nc) as tc:
        with (
            tc.tile_pool(name="adata", bufs=3) as adata,
            tc.tile_pool(name="pdata", bufs=3) as pdata,
            tc.tile_pool(name="ywork", bufs=2) as ywork,
            tc.tile_pool(name="small", bufs=1) as small,
            tc.tile_pool(name="psacc", bufs=1, space="PSUM") as psacc,
            tc.tile_pool(name="pstr", bufs=1, space="PSUM") as pstr,
        ):
            s2a = small.tile([P, len(A_CHUNKS)], f32)
            res = small.tile([P, 2], f32)

            # PSUM accumulator row: per-token sum of exp(2l) over the PE share
            ps_row = psacc.tile([1, TOK_PER_CORE], f32)

            # stationary ones for the PE vocab reduction (bf16 for 1-pass MMs)
            ones_bf = small.tile([P, 1], bf16)
            nc.any.memset(ones_bf[:], 1.0)
            ones_b1 = small.tile([1, 1], bf16)
            nc.any.memset(ones_b1[:], 1.0)

            # --- self-clocked stream: each chunk's DMA issued at its
            # consumption point in deadline order on ONE ring.  The sync
            # queue races ahead so DMAs stay ~bufs chunks ahead of compute;
            # slot semaphores throttle the stream to consumption rate.
            # (Front-loading all 16.4MB instead slams SBUF and throttles
            # ACT/DVE ~20% — the v8/v9 regression.)
            a_tiles = {}
            p_tiles = {}
            aux_sb = small.tile([P, 3 * TILES], i32)

            def issue(kind, idx):
                if kind == "G":
                    nc.sync.dma_start(out=aux_sb[:], in_=aux)
                elif kind == "A":
                    t, c0, w = A_CHUNKS[idx]
                    ach = adata.tile([P, 13312], fp8, tag="achunk")
                    nc.sync.dma_start(
                        out=ach[:, :w], in_=l8a[t * P : (t + 1) * P, c0 : c0 + w]
                    )
                    a_tiles[idx] = ach
                else:
                    ntile = PE_CHUNKS[idx]
                    col0 = sum(PE_CHUNKS[:idx]) * TOK_PER_CORE
                    pch = pdata.tile([P, 18 * TOK_PER_CORE], fp8, tag="pchunk")
                    nc.sync.dma_start(
                        out=pch[:, : ntile * TOK_PER_CORE],
                        in_=l8p[:, col0 : col0 + ntile * TOK_PER_CORE],
                    )
                    p_tiles[idx] = pch

            # first chunks + gidx issued before the small setup ops
            for ev in ISSUE_ORDER[:3]:
                issue(*ev)

            # mask: pad token <=> target==0 <=> gidx == token_index*V
            # (both staged in aux; v10-12 lesson: the on-device indirect
            # gathers for the numerator could complete as late as ~69us on
            # throttled runs — SWDGE descgen is locked out by DVE 2-port ts
            # instructions — so l_t is host-gathered into aux instead).
            mask_sb = small.tile([P, TILES], f32)
            nc.vector.tensor_tensor(
                out=mask_sb[:], in0=aux_sb[:, 0:TILES],
                in1=aux_sb[:, TILES : 2 * TILES], op=ALU.not_equal
            )
            # mask-sum is independent of everything else: do it now
            nc.vector.tensor_reduce(
                out=res[:, 1:2], in_=mask_sb[:], axis=AX.X, op=ALU.add
            )

            # --- main loop: issue remaining DMAs at their deadline slot,
            # compute each chunk as it lands.
            state = {"acts": {}, "tss": {}, "mm_done": 0}

            def compute(kind, idx):
                if kind == "G":
                    return
                if kind == "A":
                    ach = a_tiles[idx]
                    t, c0, w = A_CHUNKS[idx]
                    # in-place fp8 out is clamped garbage nothing reads; the
                    # accumulated fp32 row sums are the real output.
                    state["acts"][idx] = nc.scalar.activation(
                        out=ach[:, :w], in_=ach[:, :w], func=AF.Exp, scale=2.0,
                        accum_out=s2a[:, idx : idx + 1],
                    )
                else:
                    ntile = PE_CHUNKS[idx]
                    pch = p_tiles[idx]
                    y16 = ywork.tile([P, 18 * TOK_PER_CORE], i16, tag="y16")
                    yb = y16[:].bitcast(bf16)
                    # last chunk: split the ts in two so its first MMs start
                    # ~1.3us earlier, shortening the MM tail before the drain
                    halves = ([(0, ntile)] if idx != len(PE_CHUNKS) - 1
                              else [(0, 5), (5, ntile)])
                    for t0, t1 in halves:
                        c0, c1 = t0 * TOK_PER_CORE, t1 * TOK_PER_CORE
                        state["tss"][idx] = nc.vector.tensor_scalar(
                            out=y16[:, c0:c1], in0=pch[:, c0:c1],
                            scalar1=float(A16), scalar2=float(B16),
                            op0=ALU.mult, op1=ALU.add,
                        )
                        for k in range(t0, t1):
                            nc.tensor.matmul(
                                ps_row[:1, :],
                                ones_bf[:],
                                yb[:, k * TOK_PER_CORE : (k + 1) * TOK_PER_CORE],
                                start=(state["mm_done"] == 0),
                                stop=(state["mm_done"] == NP - 1),
                            )
                            state["mm_done"] += 1

            for n, ev in enumerate(ISSUE_ORDER):
                if n >= 3:
                    issue(*ev)
                compute(*ev)

            # --- PE-share drain: PSUM row -> bf16 SBUF -> token-major
            # [128, TILES] via 4 tiny 1-pass bf16 transpose matmuls.
            s2row = small.tile([1, TOK_PER_CORE], bf16)
            nc.vector.tensor_copy(s2row[:], ps_row[:1, :])
            ps_t = pstr.tile([P, TILES], f32)
            for t in range(TILES):
                nc.tensor.matmul(
                    ps_t[:, t : t + 1],
                    s2row[:1, t * P : (t + 1) * P],
                    ones_b1[:1, :],
                    start=True, stop=True,
                )
            s2p = small.tile([P, TILES], f32)
            nc.vector.tensor_copy(s2p[:], ps_t[:])

            # --- numerator exp, pre-masked.  Anchored MID-stream (after
            # A1 / ts3): their inputs are ready by ~9us (aux host-staged),
            # so they slot into the engines' starve gaps instead of
            # serializing after the bulk streams; anchoring them at all
            # keeps them off the FIFO heads (the v8 lesson).
            exp_lt = small.tile([P, TILES], f32)
            ei = nc.scalar.activation(
                out=exp_lt[:], in_=aux_sb[:, 2 * TILES : 3 * TILES].bitcast(f32),
                func=AF.Exp,
            )
            add_dep_helper(ei.ins, state["acts"][1].ins, sync=False,
                           reason="exp_lt mid ACT stream")
            em = small.tile([P, TILES], f32)
            emi = nc.vector.tensor_mul(em[:], exp_lt[:], mask_sb[:])
            add_dep_helper(emi.ins, state["tss"][3].ins, sync=False,
                           reason="em mid ts stream")

            # fold the split rows' accum chunks: col1 += col0 (row 0, runs
            # mid-stream — A0/A1 accums ready ~22us) and col4 += col5 (row 3
            # — A5's accum is late, so anchor after the last ts to keep it
            # off the DVE FIFO head).  s2a[:, 1:5] is then [r0, r1, r2, r3].
            fi = nc.vector.tensor_add(s2a[:, 1:2], s2a[:, 0:1], s2a[:, 1:2])
            add_dep_helper(fi.ins, state["tss"][4].ins, sync=False,
                           reason="s2a fold row0 mid ts stream")
            f2 = nc.vector.tensor_add(s2a[:, 4:5], s2a[:, 5:6], s2a[:, 4:5])
            add_dep_helper(f2.ins, state["tss"][len(PE_CHUNKS) - 1].ins,
                           sync=False, reason="s2a fold row3 after ts stream")
            s2 = small.tile([P, TILES], f32)
            nc.vector.tensor_add(s2[:], s2a[:, 1 : 1 + TILES], s2p[:])

            # rs ~= 1/sqrt(s2): fast-inverse-sqrt bit trick, no Newton step
            # (y0 rel err in [-3.4%, +1.2%] -> ~6e-5 on the loss; tol 2e-2).
            sh = small.tile([P, TILES], i32)
            nc.vector.tensor_scalar(
                out=sh[:], in0=s2[:].bitcast(i32), scalar1=1, scalar2=None,
                op0=ALU.arith_shift_right,
            )
            y0i = small.tile([P, TILES], i32)
            nc.vector.tensor_scalar(
                out=y0i[:], in0=sh[:], scalar1=-1.0, scalar2=float(0x5F3759DF),
                op0=ALU.mult, op1=ALU.add,
            )
            cosm = small.tile([P, TILES], f32)
            nc.vector.tensor_mul(cosm[:], em[:], y0i[:].bitcast(f32))

            nc.vector.tensor_reduce(
                out=res[:, 0:1], in_=cosm[:], axis=AX.X, op=ALU.add
            )
            nc.sync.dma_start(out=out, in_=res[:])

    nc.compile()
    return nc


_NC_CACHE = {}


def _get_nc():
    if "nc" not in _NC_CACHE:
        _NC_CACHE["nc"] = build_program()
    return _NC_CACHE["nc"]


def make_in_maps(logits, targets):
    """Shard full inputs into per-core input maps (host-side prep only)."""
    logits = np.asarray(logits)
    targets = np.asarray(targets)
    assert logits.shape == (B, S, V), logits.shape
    lf = np.ascontiguousarray(logits.reshape(NTOK, V).astype(np.float32, copy=False))
    l8f = lf.astype(ml_dtypes.float8_e4m3fn)
    tf = targets.reshape(NTOK).astype(np.int64)

    # token j of a core sits at (partition p = j % P, tile t = j // P)
    local_tok = (np.arange(TILES)[None, :] * P + np.arange(P)[:, None]).astype(np.int64)

    in_maps = []
    for k in range(N_CORES):
        sl = slice(k * TOK_PER_CORE, (k + 1) * TOK_PER_CORE)
        blk = lf[sl]                              # [512, V] fp32
        tk = tf[sl].reshape(TILES, P).T           # [P, TILES]
        gidx = (local_tok * V + tk).astype(np.int32)
        gbase = (local_tok * V).astype(np.int32)
        ltg = blk[local_tok, tk].astype(np.float32)   # host-gathered l_t
        aux = np.concatenate(
            [gidx, gbase, ltg.view(np.int32)], axis=1
        ).astype(np.int32)
        blk8 = l8f[sl]                            # [512, V]
        # PE share staged vocab-major: l8p[p, j*512+t] = l[t, VA + j*128 + p]
        l8p = np.ascontiguousarray(
            blk8[:, VA:].reshape(TOK_PER_CORE, NP, P).transpose(2, 1, 0)
            .reshape(P, NP * TOK_PER_CORE)
        )
        in_maps.append(
            {
                "l8a": np.ascontiguousarray(blk8[:, :VA]),
                "l8p": l8p,
                "aux": np.ascontiguousarray(aux),
            }
        )
    return in_maps


def reduce_outputs(per_core_outs):
    """Combine per-core [128, 2] partials into the final scalar loss."""
    s = 0.0
    c = 0.0
    for o in per_core_outs:
        s += float(o[:, 0].astype(np.float64).sum())
        c += float(o[:, 1].astype(np.float64).sum())
    return np.asarray(np.float32(1.0 - s / (c + EPS_MEAN)))


def run_on_device(in_maps, **kwargs):
    nc = _get_nc()
    return run_bass_kernel_spmd(nc, in_maps, core_ids=list(range(N_CORES)), **kwargs)


def kernel(logits, targets):
    in_maps = make_in_maps(logits, targets)
    res = run_on_device(in_maps)
    return reduce_outputs([r["out"] for r in res.results])
